# revision 13
# baseline (speedup 1.0000x reference)
"""Trainium2 Bass kernel for a 2-layer GCN + global mean pool + FC.

Strategy (8 NeuronCores, SPMD single NEFF):
  - Nodes (and their in-edges) partitioned by dst across 8 cores; weights
    replicated; h1 shards AllGathered between layers; pooled sums AllReduced.
  - Per 128-edge chunk, h[src] rows are fetched with dma_gather (row i ->
    partition i%128) and scatter-added via a one-hot mask matmul on the
    TensorEngine: agg[128d,64f] += S[e,d].T @ msgs[e,f] accumulating in PSUM.
  - S masks are pure 0/1 one-hots in bf16, generated in one batched DVE
    tensor_tensor op per supergather set (iota vs dst_local broadcast APs);
    the edge norm (dinv_sqrt[src]*dinv_sqrt[dst]) is folded into the msgs
    during the f32->bf16 convert of each gather tile (one batched DVE op).
  - Self-loop terms use the core's own contiguous rows (sequential DMA) and
    are fused into the per-block epilogue -- no per-edge gathers for them.
  - dma_gather indices are int16 (max 32767), so nodes are split into two
    sets A/B by their position within the owner's shard (local offset < 3200);
    gather sources are the correspondingly reordered xA/xB (host-permuted)
    and h1fullA/h1fullB. The A half of the h1 AllGather is issued as soon as
    the first 25 blocks are done, overlapping the rest of layer 1.
"""

import numpy as np
import ml_dtypes

from concourse import bacc, bass, mybir, bass_utils
from concourse.masks import make_identity
import concourse.tile as tile

N = 50000
E = 800000
F = 64          # feature width of x / h1 / h2
G = 128         # number of graphs
OUT = 8
P = 128
C = 8
NSH = N // C    # 6250 nodes per core
ABL = 3200      # A/B split point (local offset, 25 blocks)
NA = C * ABL            # rows in the A gather source (25600)
NBB = C * (NSH - ABL)   # rows in the B gather source (24400)
NB = (NSH + P - 1) // P   # 49 dst blocks per core
ABLK = ABL // P           # 25 blocks in A
SBLK = 4                  # dst blocks per supergather
NSB = (NB + SBLK - 1) // SBLK
F32 = mybir.dt.float32
BF16 = mybir.dt.bfloat16
I16 = mybir.dt.int16


def _bcast_ap(ap, dims):
    """Build a broadcast view of `ap` with explicit [step, count] dims."""
    return bass.AP(tensor=ap.tensor, offset=ap.offset, ap=dims)


def _ab_index(n):
    """Map global node id -> (set, idx-within-set) for the A/B split."""
    r, l = n // NSH, n % NSH
    s = l >= ABL
    return s, np.where(s, r * (NSH - ABL) + (l - ABL), r * ABL + l)


def _preprocess(src, dst, batch):
    """Host-side index preprocessing (pure integer/index work)."""
    src = np.asarray(src).astype(np.int64)
    dst = np.asarray(dst).astype(np.int64)
    batch = np.asarray(batch).astype(np.int64)

    deg = np.bincount(dst, minlength=N).astype(np.float32) + 1.0
    dinv = (1.0 / np.sqrt(deg)).astype(np.float32)
    norm_all = (dinv[src] * dinv[dst]).astype(np.float32)
    st_all, sidx_all = _ab_index(src)
    st_all = st_all.astype(np.int64)

    core_groups = []
    counts = np.zeros((C, NB, 2), np.int64)
    for c in range(C):
        lo = c * NSH
        m = (dst >= lo) & (dst < lo + NSH)
        es, ed, en = sidx_all[m], dst[m], norm_all[m]
        st = st_all[m]
        dloc = ed - lo
        blk = dloc >> 7
        sb = blk // SBLK
        blkin = blk - sb * SBLK
        key = (sb * 2 + st) * SBLK + blkin
        order = np.argsort(key, kind="stable")
        es, en, dloc, key = es[order], en[order], dloc[order], key[order]
        np.add.at(counts[c], (blk[order], st[order]), 1)
        core_groups.append((es, en, dloc, key))

    nch_bs = np.ceil(counts.max(axis=0) / P).astype(np.int64)  # [NB, 2]
    nch_bs = np.maximum(nch_bs, 1)

    nch_sb = np.zeros((NSB, 2), np.int64)
    for b in range(NB):
        nch_sb[b // SBLK] += nch_bs[b]
    chunk_base = {}
    idxcol_base = {}
    tot_chunks = 0
    idx_cols = [0, 0]
    for sbi in range(NSB):
        for s in range(2):
            chunk_base[(sbi, s)] = tot_chunks
            tot_chunks += int(nch_sb[sbi, s])
            idxcol_base[(sbi, s)] = idx_cols[s]
            idx_cols[s] += int(nch_sb[sbi, s]) * (P // 16)
    off_in_tile = np.zeros((NB, 2), np.int64)
    for sbi in range(NSB):
        run = [0, 0]
        for b in range(sbi * SBLK, min((sbi + 1) * SBLK, NB)):
            for s in range(2):
                off_in_tile[b, s] = run[s]
                run[s] += int(nch_bs[b, s])

    plan = dict(nch_bs=nch_bs, nch_sb=nch_sb, chunk_base=chunk_base,
                idxcol_base=idxcol_base, off_in_tile=off_in_tile,
                tot_chunks=tot_chunks, idx_cols=idx_cols)

    per_core = []
    for c in range(C):
        es, en, dloc, key = core_groups[c]
        bounds = np.searchsorted(key, np.arange(NSB * 2 * SBLK + 1))
        idx_parts = [[], []]
        dl_parts = []
        nm_parts = []
        for sbi in range(NSB):
            for s in range(2):
                for b in range(sbi * SBLK, min((sbi + 1) * SBLK, NB)):
                    k = (sbi * 2 + s) * SBLK + (b - sbi * SBLK)
                    g0, g1 = bounds[k], bounds[k + 1]
                    n = g1 - g0
                    want = int(nch_bs[b, s]) * P
                    assert n <= want
                    gi = np.zeros(want, np.int64)
                    gd = np.zeros(want, np.int64)
                    gn = np.zeros(want, np.float32)
                    gi[:n] = es[g0:g1]
                    gd[:n] = dloc[g0:g1] - (b << 7)
                    gn[:n] = en[g0:g1]
                    idx_parts[s].append(gi)
                    dl_parts.append(gd)
                    nm_parts.append(gn)
        dstloc = np.concatenate(dl_parts).reshape(-1, P).T
        normv = np.concatenate(nm_parts).reshape(-1, P).T.astype(np.float32)
        idx = []
        for s in range(2):
            stk = np.concatenate(idx_parts[s]).astype(np.int16)
            idx.append(np.tile(stk.reshape(-1, 16).T, (8, 1)))
        batchloc = np.full((P, NB), -1.0, np.float32)
        full = np.full(NB * P, -1.0, np.float32)
        full[:NSH] = batch[c * NSH:(c + 1) * NSH]
        batchloc[:, :] = full.reshape(NB, P).T
        selfw = np.zeros(NB * P, np.float32)
        selfw[:NSH] = 1.0 / deg[c * NSH:(c + 1) * NSH]
        selfw = selfw.reshape(NB, P).T.copy()
        per_core.append(dict(
            idx0=idx[0], idx1=idx[1],
            dstloc=dstloc.astype(ml_dtypes.bfloat16),
            normv=normv, batchloc=batchloc, selfw=selfw))

    cnt = np.bincount(batch, minlength=G).astype(np.float32)
    invc = (1.0 / np.maximum(cnt, 1.0)).astype(np.float32)
    return plan, per_core, invc


def _build(plan):
    """Build the SPMD Bass program (identical for all cores)."""
    nch_bs = plan["nch_bs"]
    nch_sb = plan["nch_sb"]
    chunk_base = plan["chunk_base"]
    idxcol_base = plan["idxcol_base"]
    off_in_tile = plan["off_in_tile"]
    NCH = plan["tot_chunks"]
    icols = plan["idx_cols"]

    nc = bacc.Bacc("TRN2", target_bir_lowering=False, debug=False,
                   num_devices=C, num_swdge_queues=4)

    xA = nc.dram_tensor("xA", [NA, F], F32, kind="ExternalInput")
    xB = nc.dram_tensor("xB", [NBB, F], F32, kind="ExternalInput")
    xown = nc.dram_tensor("xown", [NSH, F], F32, kind="ExternalInput")
    idx0 = nc.dram_tensor("idx0", [P, icols[0]], I16, kind="ExternalInput")
    idx1 = nc.dram_tensor("idx1", [P, icols[1]], I16, kind="ExternalInput")
    dstloc = nc.dram_tensor("dstloc", [P, NCH], BF16, kind="ExternalInput")
    normv = nc.dram_tensor("normv", [P, NCH], F32, kind="ExternalInput")
    batchloc = nc.dram_tensor("batchloc", [P, NB], F32, kind="ExternalInput")
    selfw_in = nc.dram_tensor("selfw", [P, NB], F32, kind="ExternalInput")
    iota_in = nc.dram_tensor("iota", [P, P], BF16, kind="ExternalInput")
    W1 = nc.dram_tensor("W1", [F, F], F32, kind="ExternalInput")
    W2 = nc.dram_tensor("W2", [F, F], F32, kind="ExternalInput")
    Wfc = nc.dram_tensor("Wfc", [F, OUT], F32, kind="ExternalInput")
    b1b = nc.dram_tensor("b1b", [P, F], F32, kind="ExternalInput")
    b2b = nc.dram_tensor("b2b", [P, F], F32, kind="ExternalInput")
    bfcb = nc.dram_tensor("bfcb", [P, OUT], F32, kind="ExternalInput")
    invc_in = nc.dram_tensor("invc", [F, G], F32, kind="ExternalInput")
    out = nc.dram_tensor("out", [G, OUT], F32, kind="ExternalOutput")

    gq = [0]  # rotating swdge queue counter

    with tile.TileContext(nc) as tc:
        with (
            tc.tile_pool(name="const", bufs=1) as cp,
            tc.tile_pool(name="gpool", bufs=2) as gp,
            tc.tile_pool(name="mpool", bufs=3) as mp,
            tc.tile_pool(name="spool", bufs=3) as sp,
            tc.tile_pool(name="epool", bufs=3) as ep,
            tc.tile_pool(name="psA", bufs=2, space="PSUM") as psA,
            tc.tile_pool(name="psB", bufs=1, space="PSUM") as psB,
            tc.tile_pool(name="dram", bufs=1, space="DRAM") as dram,
        ):
            # ---- constants / metadata loads ----
            iota_sb = cp.tile([P, P], BF16, tag="iota")
            nc.sync.dma_start(iota_sb[:], iota_in[:])
            iota_f32 = cp.tile([P, P], F32, tag="iota32")
            nc.vector.tensor_copy(iota_f32[:], iota_sb[:])
            ident = cp.tile([P, P], F32, tag="ident")
            make_identity(nc, ident[:])
            idx_sb = [cp.tile([P, icols[0]], I16, tag="idx0", name="idx_sb0"),
                      cp.tile([P, icols[1]], I16, tag="idx1", name="idx_sb1")]
            nc.sync.dma_start(idx_sb[0][:], idx0[:])
            nc.sync.dma_start(idx_sb[1][:], idx1[:])
            dl_sb = cp.tile([P, NCH], BF16, tag="dstloc")
            nc.sync.dma_start(dl_sb[:], dstloc[:])
            nm_sb = cp.tile([P, NCH], F32, tag="normv")
            nc.sync.dma_start(nm_sb[:], normv[:])
            bl_sb = cp.tile([P, NB], F32, tag="batchloc")
            nc.sync.dma_start(bl_sb[:], batchloc[:])
            sw_sb = cp.tile([P, NB], F32, tag="selfw")
            nc.sync.dma_start(sw_sb[:], selfw_in[:])
            W1_sb = cp.tile([F, F], F32, tag="W1")
            nc.sync.dma_start(W1_sb[:], W1[:])
            W2_sb = cp.tile([F, F], F32, tag="W2")
            nc.sync.dma_start(W2_sb[:], W2[:])
            Wfc_sb = cp.tile([F, OUT], F32, tag="Wfc")
            nc.sync.dma_start(Wfc_sb[:], Wfc[:])
            b1_sb = cp.tile([P, F], F32, tag="b1b")
            nc.sync.dma_start(b1_sb[:], b1b[:])
            b2_sb = cp.tile([P, F], F32, tag="b2b")
            nc.sync.dma_start(b2_sb[:], b2b[:])
            bfc_sb = cp.tile([P, OUT], F32, tag="bfcb")
            nc.sync.dma_start(bfc_sb[:], bfcb[:])
            invc_sb = cp.tile([F, G], F32, tag="invc")
            nc.sync.dma_start(invc_sb[:], invc_in[:])

            h1shardA = dram.tile([ABL, F], F32)
            h1shardB = dram.tile([NSH - ABL, F], F32)
            h1fullA = dram.tile([NA, F], F32, addr_space="Shared")
            h1fullB = dram.tile([NBB, F], F32, addr_space="Shared")
            pool_in = dram.tile([F, G], F32)
            pool_out = dram.tile([F, G], F32, addr_space="Shared")

            pool_ps = psB.tile([F, G], F32, tag="pool")

            def gather(t, src_ap, idx_tile, icol0, nidx):
                q = gq[0] % 4
                gq[0] += 1
                nc.gpsimd.dma_gather(
                    t[:], src_ap, idx_tile[:, icol0:icol0 + nidx // 16],
                    nidx, nidx, F,
                    single_packet=False, queue_num=q,
                )

            NBF = NB - 1          # full 128-row blocks in a shard
            LASTR = NSH - NBF * P  # rows in the last partial block

            def conv_layer(srcsAB, own_parts, W_sb, bb_sb, sink):
                # own rows for self-loop term: [128, NB, 64]
                x_own = ep.tile([P, NB, F], F32, tag="x_own", bufs=2)
                nc.vector.memset(x_own[:, NBF, :], 0.0)
                for (ap_src, b0, nrow) in own_parts:
                    nfull = nrow // P
                    if nfull:
                        nc.sync.dma_start(
                            x_own[:, b0:b0 + nfull, :],
                            ap_src[:nfull * P, :].rearrange("(b p) f -> p b f", p=P),
                        )
                    rem = nrow - nfull * P
                    if rem:
                        nc.sync.dma_start(
                            x_own[:rem, b0 + nfull, :],
                            ap_src[nfull * P:nrow, :],
                        )
                for sbi in range(NSB):
                    mt = {}
                    St = {}
                    for s in range(2):
                        nch = int(nch_sb[sbi, s])
                        if nch == 0:
                            continue
                        gt = gp.tile([P, nch, F], F32, tag=f"g{s}")
                        nidx = nch * P
                        gather(gt, srcsAB[s], idx_sb[s], idxcol_base[(sbi, s)], nidx)
                        cb = chunk_base[(sbi, s)]
                        # fused norm-scale + f32->bf16 convert, one op per tile
                        m_t = mp.tile([P, nch, F], BF16, tag=f"m{s}")
                        nmap = nm_sb[:, cb:cb + nch]
                        nc.vector.tensor_tensor(
                            out=m_t[:],
                            in0=gt[:],
                            in1=_bcast_ap(nmap, [nmap.ap[0], [nmap.ap[1][0], nch], [0, F]]),
                            op=mybir.AluOpType.mult,
                        )
                        mt[s] = m_t
                        # batched one-hot S for the whole supergather set
                        S_t = sp.tile([P, nch, P], BF16, tag=f"S{s}")
                        dmap = dl_sb[:, cb:cb + nch]
                        nc.vector.tensor_tensor(
                            out=S_t[:],
                            in0=_bcast_ap(iota_sb[:], [iota_sb[:].ap[0], [0, nch], [1, P]]),
                            in1=_bcast_ap(dmap, [dmap.ap[0], [dmap.ap[1][0], nch], [0, P]]),
                            op=mybir.AluOpType.is_equal,
                        )
                        St[s] = S_t
                    for b in range(sbi * SBLK, min((sbi + 1) * SBLK, NB)):
                        agg_ps = psA.tile([P, F], F32, tag="agg")
                        tot = int(nch_bs[b, 0] + nch_bs[b, 1])
                        done = 0
                        for s in range(2):
                            nch = int(nch_bs[b, s])
                            if nch == 0:
                                continue
                            off = int(off_in_tile[b, s])
                            for ci in range(nch):
                                nc.tensor.matmul(
                                    agg_ps[:], lhsT=St[s][:, off + ci, :],
                                    rhs=mt[s][:, off + ci, :],
                                    start=(done == 0), stop=(done == tot - 1),
                                )
                                done += 1
                        # epilogue: h = tanh((agg + selfw*own) @ W + b)
                        tmp = ep.tile([P, F], F32, tag="tmp")
                        nc.vector.tensor_scalar(
                            tmp[:], x_own[:, b, :], sw_sb[:, b:b + 1], None,
                            mybir.AluOpType.mult,
                        )
                        agg_sb = ep.tile([P, F], F32, tag="agg_sb")
                        nc.vector.tensor_add(agg_sb[:], agg_ps[:], tmp[:])
                        trp = psA.tile([F, P], F32, tag="tr")
                        nc.tensor.transpose(trp[:], agg_sb[:], ident[:])
                        aggT = ep.tile([F, P], F32, tag="aggT")
                        nc.vector.tensor_copy(aggT[:], trp[:])
                        h_ps = psA.tile([P, F], F32, tag="h")
                        nc.tensor.matmul(h_ps[:], lhsT=aggT[:], rhs=W_sb[:],
                                         start=True, stop=True)
                        h_sb = ep.tile([P, F], F32, tag="h_sb")
                        nc.vector.tensor_add(h_sb[:], h_ps[:], bb_sb[:])
                        nc.scalar.activation(h_sb[:], h_sb[:],
                                             mybir.ActivationFunctionType.Tanh)
                        sink(b, h_sb)

            def sink1(b, h_sb):
                if b < ABLK:
                    r0 = b * P
                    nc.sync.dma_start(h1shardA[r0:r0 + P, :], h_sb[:])
                    if b == ABLK - 1:
                        nc.gpsimd.collective_compute(
                            "AllGather", mybir.AluOpType.bypass,
                            ins=[h1shardA.opt()], outs=[h1fullA.opt()],
                            replica_groups=[list(range(C))],
                        )
                else:
                    r0 = (b - ABLK) * P
                    rows = min(P, (NSH - ABL) - r0)
                    nc.sync.dma_start(h1shardB[r0:r0 + rows, :], h_sb[:rows, :])
                    if b == NB - 1:
                        nc.gpsimd.collective_compute(
                            "AllGather", mybir.AluOpType.bypass,
                            ins=[h1shardB.opt()], outs=[h1fullB.opt()],
                            replica_groups=[list(range(C))],
                        )

            def sink2(b, h_sb):
                Sp = sp.tile([P, G], F32, tag="Spool")
                nc.vector.tensor_scalar(
                    Sp[:], iota_f32[:], bl_sb[:, b:b + 1], None,
                    mybir.AluOpType.is_equal,
                )
                nc.tensor.matmul(pool_ps[:], lhsT=h_sb[:], rhs=Sp[:],
                                 start=(b == 0), stop=(b == NB - 1),
                                 skip_group_check=True)

            conv_layer((xA[:], xB[:]), [(xown[:], 0, NSH)], W1_sb, b1_sb, sink1)
            conv_layer((h1fullA[:], h1fullB[:]),
                       [(h1shardA[:], 0, ABL), (h1shardB[:], ABLK, NSH - ABL)],
                       W2_sb, b2_sb, sink2)

            # ---- pooled tail ----
            poolT = ep.tile([F, G], F32, tag="poolT")
            nc.vector.tensor_copy(poolT[:], pool_ps[:])
            nc.sync.dma_start(pool_in[:], poolT[:])
            nc.gpsimd.collective_compute(
                "AllReduce", mybir.AluOpType.add,
                ins=[pool_in.opt()], outs=[pool_out.opt()],
                replica_groups=[list(range(C))],
            )
            poolR = ep.tile([F, G], F32, tag="poolR")
            nc.sync.dma_start(poolR[:], pool_out[:])
            nc.vector.tensor_mul(poolR[:], poolR[:], invc_sb[:])
            fc_ps = psB.tile([G, OUT], F32, tag="fc")
            nc.tensor.matmul(fc_ps[:], lhsT=poolR[:], rhs=Wfc_sb[:],
                             start=True, stop=True)
            out_sb = ep.tile([G, OUT], F32, tag="out_sb")
            nc.vector.tensor_add(out_sb[:], fc_ps[:], bfc_sb[:])
            nc.sync.dma_start(out[:], out_sb[:])

    nc.compile()
    return nc


def _in_maps(plan, per_core, invc, x, W1, b1, W2, b2, Wfc, bfc):
    iota = np.tile(np.arange(P, dtype=np.float32), (P, 1)).astype(ml_dtypes.bfloat16)
    xf = np.ascontiguousarray(np.asarray(x, np.float32))
    xr = xf.reshape(C, NSH, F)
    xA = np.ascontiguousarray(xr[:, :ABL, :].reshape(NA, F))
    xB = np.ascontiguousarray(xr[:, ABL:, :].reshape(NBB, F))
    shared = dict(
        xA=xA, xB=xB,
        iota=iota,
        W1=np.ascontiguousarray(np.asarray(W1, np.float32)),
        W2=np.ascontiguousarray(np.asarray(W2, np.float32)),
        Wfc=np.ascontiguousarray(np.asarray(Wfc, np.float32)),
        b1b=np.tile(np.asarray(b1, np.float32), (P, 1)),
        b2b=np.tile(np.asarray(b2, np.float32), (P, 1)),
        bfcb=np.tile(np.asarray(bfc, np.float32), (P, 1)),
        invc=np.tile(invc, (F, 1)),
    )
    maps = []
    for c in range(C):
        m = dict(shared)
        m.update(per_core[c])
        m["xown"] = xf[c * NSH:(c + 1) * NSH]
        maps.append({k: np.ascontiguousarray(v) for k, v in m.items()})
    return maps


_RUN_KWARGS = {}


def kernel(x, src, dst, batch, W1, b1, W2, b2, Wfc, bfc):
    plan, per_core, invc = _preprocess(src, dst, batch)
    nc = _build(plan)
    maps = _in_maps(plan, per_core, invc, x, W1, b1, W2, b2, Wfc, bfc)
    res = bass_utils.run_bass_kernel_spmd(
        nc, maps, core_ids=list(range(C)), **_RUN_KWARGS
    )
    kernel.last_results = res
    return np.asarray(res.results[0]["out"], np.float32)


# revision 18
# speedup vs baseline: 1.4496x; 1.4496x over previous
"""Trainium2 Bass kernel for a 2-layer GCN + global mean pool + FC.

Strategy (8 NeuronCores, SPMD single NEFF):
  - Nodes (and their in-edges) partitioned by dst across 8 cores; weights
    replicated; h1 shards AllGathered between layers; pooled sums AllReduced.
  - Per 128-edge chunk, h[src] rows are fetched with dma_gather (row i ->
    partition i%128) and scatter-added via a one-hot mask matmul on the
    TensorEngine: agg[128d,64f] += S[e,d].T @ msgs[e,f] accumulating in PSUM.
  - S masks are pure 0/1 one-hots in bf16, generated in one batched DVE
    tensor_tensor op per supergather set (iota vs dst_local broadcast APs);
    the edge norm (dinv_sqrt[src]*dinv_sqrt[dst]) is folded into the msgs
    during the f32->bf16 convert of each gather tile (one batched DVE op).
  - Self-loop terms use the core's own contiguous rows (sequential DMA) and
    are fused into the per-block epilogue -- no per-edge gathers for them.
  - dma_gather indices are int16 (max 32767), so nodes are split into two
    sets A/B by their position within the owner's shard (local offset < 3200);
    gather sources are the correspondingly reordered xA/xB (host-permuted)
    and h1fullA/h1fullB. The A half of the h1 AllGather is issued as soon as
    the first 25 blocks are done, overlapping the rest of layer 1.
"""

import numpy as np
import ml_dtypes

from concourse import bacc, bass, mybir, bass_utils
from concourse.masks import make_identity
import concourse.tile as tile

N = 50000
E = 800000
F = 64          # feature width of x / h1 / h2
G = 128         # number of graphs
OUT = 8
P = 128
C = 8
NSH = N // C    # 6250 nodes per core
ABL = 3200      # A/B split point (local offset, 25 blocks)
NA = C * ABL            # rows in the A gather source (25600)
NBB = C * (NSH - ABL)   # rows in the B gather source (24400)
NB = (NSH + P - 1) // P   # 49 dst blocks per core
ABLK = ABL // P           # 25 blocks in A
SBLK = 4                  # dst blocks per supergather
NSB = (NB + SBLK - 1) // SBLK
F32 = mybir.dt.float32
BF16 = mybir.dt.bfloat16
I16 = mybir.dt.int16


def _bcast_ap(ap, dims):
    """Build a broadcast view of `ap` with explicit [step, count] dims."""
    return bass.AP(tensor=ap.tensor, offset=ap.offset, ap=dims)


def _ab_index(n):
    """Map global node id -> (set, idx-within-set) for the A/B split."""
    r, l = n // NSH, n % NSH
    s = l >= ABL
    return s, np.where(s, r * (NSH - ABL) + (l - ABL), r * ABL + l)


def _preprocess(src, dst, batch):
    """Host-side index preprocessing (pure integer/index work)."""
    src = np.asarray(src).astype(np.int64)
    dst = np.asarray(dst).astype(np.int64)
    batch = np.asarray(batch).astype(np.int64)

    deg = np.bincount(dst, minlength=N).astype(np.float32) + 1.0
    dinv = (1.0 / np.sqrt(deg)).astype(np.float32)
    norm_all = (dinv[src] * dinv[dst]).astype(np.float32)
    st_all, sidx_all = _ab_index(src)
    st_all = st_all.astype(np.int64)

    core_groups = []
    counts = np.zeros((C, NB, 2), np.int64)
    for c in range(C):
        lo = c * NSH
        m = (dst >= lo) & (dst < lo + NSH)
        es, ed, en = sidx_all[m], dst[m], norm_all[m]
        st = st_all[m]
        dloc = ed - lo
        blk = dloc >> 7
        sb = blk // SBLK
        blkin = blk - sb * SBLK
        key = (sb * 2 + st) * SBLK + blkin
        order = np.argsort(key, kind="stable")
        es, en, dloc, key = es[order], en[order], dloc[order], key[order]
        np.add.at(counts[c], (blk[order], st[order]), 1)
        core_groups.append((es, en, dloc, key))

    nch_bs = np.ceil(counts.max(axis=0) / P).astype(np.int64)  # [NB, 2]
    nch_bs = np.maximum(nch_bs, 1)

    nch_sb = np.zeros((NSB, 2), np.int64)
    for b in range(NB):
        nch_sb[b // SBLK] += nch_bs[b]
    chunk_base = {}
    idxcol_base = {}
    tot_chunks = 0
    idx_cols = [0, 0]
    for sbi in range(NSB):
        for s in range(2):
            chunk_base[(sbi, s)] = tot_chunks
            tot_chunks += int(nch_sb[sbi, s])
            idxcol_base[(sbi, s)] = idx_cols[s]
            idx_cols[s] += int(nch_sb[sbi, s]) * (P // 16)
    off_in_tile = np.zeros((NB, 2), np.int64)
    for sbi in range(NSB):
        run = [0, 0]
        for b in range(sbi * SBLK, min((sbi + 1) * SBLK, NB)):
            for s in range(2):
                off_in_tile[b, s] = run[s]
                run[s] += int(nch_bs[b, s])

    plan = dict(nch_bs=nch_bs, nch_sb=nch_sb, chunk_base=chunk_base,
                idxcol_base=idxcol_base, off_in_tile=off_in_tile,
                tot_chunks=tot_chunks, idx_cols=idx_cols)

    per_core = []
    for c in range(C):
        es, en, dloc, key = core_groups[c]
        bounds = np.searchsorted(key, np.arange(NSB * 2 * SBLK + 1))
        idx_parts = [[], []]
        dl_parts = []
        nm_parts = []
        for sbi in range(NSB):
            for s in range(2):
                for b in range(sbi * SBLK, min((sbi + 1) * SBLK, NB)):
                    k = (sbi * 2 + s) * SBLK + (b - sbi * SBLK)
                    g0, g1 = bounds[k], bounds[k + 1]
                    n = g1 - g0
                    want = int(nch_bs[b, s]) * P
                    assert n <= want
                    gi = np.zeros(want, np.int64)
                    gd = np.zeros(want, np.int64)
                    gn = np.zeros(want, np.float32)
                    gi[:n] = es[g0:g1]
                    gd[:n] = dloc[g0:g1] - (b << 7)
                    gn[:n] = en[g0:g1]
                    idx_parts[s].append(gi)
                    dl_parts.append(gd)
                    nm_parts.append(gn)
        dstloc = np.concatenate(dl_parts).reshape(-1, P).T
        normv = np.concatenate(nm_parts).reshape(-1, P).T.astype(np.float32)
        idx = []
        for s in range(2):
            stk = np.concatenate(idx_parts[s]).astype(np.int16)
            idx.append(np.tile(stk.reshape(-1, 16).T, (8, 1)))
        batchloc = np.full((P, NB), -1.0, np.float32)
        full = np.full(NB * P, -1.0, np.float32)
        full[:NSH] = batch[c * NSH:(c + 1) * NSH]
        batchloc[:, :] = full.reshape(NB, P).T
        selfw = np.zeros(NB * P, np.float32)
        selfw[:NSH] = 1.0 / deg[c * NSH:(c + 1) * NSH]
        selfw = selfw.reshape(NB, P).T.copy()
        per_core.append(dict(
            idx0=idx[0], idx1=idx[1],
            dstloc=dstloc.astype(ml_dtypes.bfloat16),
            normv=normv, batchloc=batchloc.astype(ml_dtypes.bfloat16), selfw=selfw))

    cnt = np.bincount(batch, minlength=G).astype(np.float32)
    invc = (1.0 / np.maximum(cnt, 1.0)).astype(np.float32)
    return plan, per_core, invc


def _build(plan):
    """Build the SPMD Bass program (identical for all cores)."""
    nch_bs = plan["nch_bs"]
    nch_sb = plan["nch_sb"]
    chunk_base = plan["chunk_base"]
    idxcol_base = plan["idxcol_base"]
    off_in_tile = plan["off_in_tile"]
    NCH = plan["tot_chunks"]
    icols = plan["idx_cols"]

    nc = bacc.Bacc("TRN2", target_bir_lowering=False, debug=False,
                   num_devices=C, num_swdge_queues=4)

    xA = nc.dram_tensor("xA", [NA, F], F32, kind="ExternalInput")
    xB = nc.dram_tensor("xB", [NBB, F], F32, kind="ExternalInput")
    xown = nc.dram_tensor("xown", [NSH, F], F32, kind="ExternalInput")
    idx0 = nc.dram_tensor("idx0", [P, icols[0]], I16, kind="ExternalInput")
    idx1 = nc.dram_tensor("idx1", [P, icols[1]], I16, kind="ExternalInput")
    dstloc = nc.dram_tensor("dstloc", [P, NCH], BF16, kind="ExternalInput")
    normv = nc.dram_tensor("normv", [P, NCH], F32, kind="ExternalInput")
    batchloc = nc.dram_tensor("batchloc", [P, NB], BF16, kind="ExternalInput")
    selfw_in = nc.dram_tensor("selfw", [P, NB], F32, kind="ExternalInput")
    iota_in = nc.dram_tensor("iota", [P, P], BF16, kind="ExternalInput")
    W1 = nc.dram_tensor("W1", [F, F], F32, kind="ExternalInput")
    W2 = nc.dram_tensor("W2", [F, F], F32, kind="ExternalInput")
    Wfc = nc.dram_tensor("Wfc", [F, OUT], F32, kind="ExternalInput")
    b1b = nc.dram_tensor("b1b", [P, F], F32, kind="ExternalInput")
    b2b = nc.dram_tensor("b2b", [P, F], F32, kind="ExternalInput")
    bfcb = nc.dram_tensor("bfcb", [P, OUT], F32, kind="ExternalInput")
    invc_in = nc.dram_tensor("invc", [F, G], F32, kind="ExternalInput")
    out = nc.dram_tensor("out", [G, OUT], F32, kind="ExternalOutput")

    gq = [0]  # rotating swdge queue counter

    with tile.TileContext(nc) as tc:
        with (
            tc.tile_pool(name="const", bufs=1) as cp,
            tc.tile_pool(name="gpool", bufs=2) as gp,
            tc.tile_pool(name="mpool", bufs=3) as mp,
            tc.tile_pool(name="spool", bufs=3) as sp,
            tc.tile_pool(name="epool", bufs=3) as ep,
            tc.tile_pool(name="psA", bufs=2, space="PSUM") as psA,
            tc.tile_pool(name="psB", bufs=1, space="PSUM") as psB,
            tc.tile_pool(name="dram", bufs=1, space="DRAM") as dram,
        ):
            # ---- constants / metadata loads ----
            iota_sb = cp.tile([P, P], BF16, tag="iota")
            nc.sync.dma_start(iota_sb[:], iota_in[:])
            ident = cp.tile([P, P], F32, tag="ident")
            make_identity(nc, ident[:])
            idx_sb = [cp.tile([P, icols[0]], I16, tag="idx0", name="idx_sb0"),
                      cp.tile([P, icols[1]], I16, tag="idx1", name="idx_sb1")]
            nc.sync.dma_start(idx_sb[0][:], idx0[:])
            nc.sync.dma_start(idx_sb[1][:], idx1[:])
            dl_sb = cp.tile([P, NCH], BF16, tag="dstloc")
            nc.sync.dma_start(dl_sb[:], dstloc[:])
            nm_sb = cp.tile([P, NCH], F32, tag="normv")
            nc.sync.dma_start(nm_sb[:], normv[:])
            bl_sb = cp.tile([P, NB], BF16, tag="batchloc")
            nc.sync.dma_start(bl_sb[:], batchloc[:])
            sw_sb = cp.tile([P, NB], F32, tag="selfw")
            nc.sync.dma_start(sw_sb[:], selfw_in[:])
            W1_sb = cp.tile([F, F], F32, tag="W1")
            nc.sync.dma_start(W1_sb[:], W1[:])
            W2_sb = cp.tile([F, F], F32, tag="W2")
            nc.sync.dma_start(W2_sb[:], W2[:])
            Wfc_sb = cp.tile([F, OUT], F32, tag="Wfc")
            nc.sync.dma_start(Wfc_sb[:], Wfc[:])
            b1_sb = cp.tile([P, F], F32, tag="b1b")
            nc.sync.dma_start(b1_sb[:], b1b[:])
            b2_sb = cp.tile([P, F], F32, tag="b2b")
            nc.sync.dma_start(b2_sb[:], b2b[:])
            bfc_sb = cp.tile([P, OUT], F32, tag="bfcb")
            nc.sync.dma_start(bfc_sb[:], bfcb[:])
            invc_sb = cp.tile([F, G], F32, tag="invc")
            nc.sync.dma_start(invc_sb[:], invc_in[:])

            h1shardA = dram.tile([ABL, F], F32)
            h1shardB = dram.tile([NSH - ABL, F], F32)
            h1fullA = dram.tile([NA, F], F32, addr_space="Shared")
            h1fullB = dram.tile([NBB, F], F32, addr_space="Shared")
            pool_in = dram.tile([F, G], F32)
            pool_out = dram.tile([F, G], F32, addr_space="Shared")

            pool_ps = psB.tile([F, G], F32, tag="pool")

            def gather(t, src_ap, idx_tile, icol0, nidx):
                q = gq[0] % 4
                gq[0] += 1
                nc.gpsimd.dma_gather(
                    t[:], src_ap, idx_tile[:, icol0:icol0 + nidx // 16],
                    nidx, nidx, F,
                    single_packet=False, queue_num=q,
                )

            NBF = NB - 1          # full 128-row blocks in a shard
            LASTR = NSH - NBF * P  # rows in the last partial block

            def conv_layer(srcsAB, own_parts, W_sb, bb_sb, sink, h_dt):
                # own rows for self-loop term: [128, NB, 64]
                x_own = ep.tile([P, NB, F], F32, tag="x_own", bufs=1)
                nc.vector.memset(x_own[:, NBF, :], 0.0)
                for (ap_src, b0, nrow) in own_parts:
                    nfull = nrow // P
                    if nfull:
                        nc.sync.dma_start(
                            x_own[:, b0:b0 + nfull, :],
                            ap_src[:nfull * P, :].rearrange("(b p) f -> p b f", p=P),
                        )
                    rem = nrow - nfull * P
                    if rem:
                        nc.sync.dma_start(
                            x_own[:rem, b0 + nfull, :],
                            ap_src[nfull * P:nrow, :],
                        )
                # batched self-loop term: tmp_all[:, b, :] = x_own[:, b, :]*selfw[:, b]
                tmp_all = ep.tile([P, NB, F], F32, tag="tmp_all", bufs=1)
                swm = sw_sb[:, :]
                nc.vector.tensor_tensor(
                    out=tmp_all[:],
                    in0=x_own[:],
                    in1=_bcast_ap(swm, [swm.ap[0], [swm.ap[1][0], NB], [0, F]]),
                    op=mybir.AluOpType.mult,
                )
                for sbi in range(NSB):
                    mt = {}
                    St = {}
                    for s in range(2):
                        nch = int(nch_sb[sbi, s])
                        if nch == 0:
                            continue
                        gt = gp.tile([P, nch, F], F32, tag=f"g{s}")
                        nidx = nch * P
                        gather(gt, srcsAB[s], idx_sb[s], idxcol_base[(sbi, s)], nidx)
                        cb = chunk_base[(sbi, s)]
                        # fused norm-scale + f32->bf16 convert, one op per tile
                        m_t = mp.tile([P, nch, F], BF16, tag=f"m{s}")
                        nmap = nm_sb[:, cb:cb + nch]
                        nc.vector.tensor_tensor(
                            out=m_t[:],
                            in0=gt[:],
                            in1=_bcast_ap(nmap, [nmap.ap[0], [nmap.ap[1][0], nch], [0, F]]),
                            op=mybir.AluOpType.mult,
                        )
                        mt[s] = m_t
                        # batched one-hot S for the whole supergather set
                        S_t = sp.tile([P, nch, P], BF16, tag=f"S{s}")
                        dmap = dl_sb[:, cb:cb + nch]
                        nc.vector.tensor_tensor(
                            out=S_t[:],
                            in0=_bcast_ap(iota_sb[:], [iota_sb[:].ap[0], [0, nch], [1, P]]),
                            in1=_bcast_ap(dmap, [dmap.ap[0], [dmap.ap[1][0], nch], [0, P]]),
                            op=mybir.AluOpType.is_equal,
                        )
                        St[s] = S_t
                    for b in range(sbi * SBLK, min((sbi + 1) * SBLK, NB)):
                        agg_ps = psA.tile([P, F], F32, tag="agg")
                        tot = int(nch_bs[b, 0] + nch_bs[b, 1])
                        done = 0
                        for s in range(2):
                            nch = int(nch_bs[b, s])
                            if nch == 0:
                                continue
                            off = int(off_in_tile[b, s])
                            for ci in range(nch):
                                nc.tensor.matmul(
                                    agg_ps[:], lhsT=St[s][:, off + ci, :],
                                    rhs=mt[s][:, off + ci, :],
                                    start=(done == 0), stop=(done == tot - 1),
                                )
                                done += 1
                        # epilogue: h = tanh((agg + selfw*own) @ W + b)
                        agg_sb = ep.tile([P, F], F32, tag="agg_sb", bufs=6)
                        nc.vector.tensor_add(agg_sb[:], agg_ps[:], tmp_all[:, b, :])
                        trp = psA.tile([F, P], F32, tag="tr")
                        nc.tensor.transpose(trp[:], agg_sb[:], ident[:])
                        aggT = ep.tile([F, P], F32, tag="aggT", bufs=6)
                        nc.vector.tensor_copy(aggT[:], trp[:])
                        h_ps = psA.tile([P, F], F32, tag="h")
                        nc.tensor.matmul(h_ps[:], lhsT=aggT[:], rhs=W_sb[:],
                                         start=True, stop=True)
                        hf_sb = ep.tile([P, F], F32, tag="hf_sb", bufs=6)
                        nc.vector.tensor_add(hf_sb[:], h_ps[:], bb_sb[:])
                        h_sb = ep.tile([P, F], h_dt, tag="h_sb", bufs=6)
                        nc.scalar.activation(h_sb[:], hf_sb[:],
                                             mybir.ActivationFunctionType.Tanh)
                        sink(b, h_sb)

            def sink1(b, h_sb):
                if b < ABLK:
                    r0 = b * P
                    nc.sync.dma_start(h1shardA[r0:r0 + P, :], h_sb[:])
                else:
                    r0 = (b - ABLK) * P
                    rows = min(P, (NSH - ABL) - r0)
                    nc.sync.dma_start(h1shardB[r0:r0 + rows, :], h_sb[:rows, :])

            def sink2(b, h_sb):
                nc.tensor.matmul(pool_ps[:], lhsT=h_sb[:], rhs=Sp_all[:, b, :],
                                 start=(b == 0), stop=(b == NB - 1),
                                 skip_group_check=True)

            conv_layer((xA[:], xB[:]), [(xown[:], 0, NSH)], W1_sb, b1_sb,
                       sink1, F32)
            nc.gpsimd.collective_compute(
                "AllGather", mybir.AluOpType.bypass,
                ins=[h1shardA.opt()], outs=[h1fullA.opt()],
                replica_groups=[list(range(C))],
            )
            nc.gpsimd.collective_compute(
                "AllGather", mybir.AluOpType.bypass,
                ins=[h1shardB.opt()], outs=[h1fullB.opt()],
                replica_groups=[list(range(C))],
            )
            # batched pool one-hots for all 49 blocks
            Sp_all = cp.tile([P, NB, G], BF16, tag="Sp_all")
            blm = bl_sb[:, :]
            nc.vector.tensor_tensor(
                out=Sp_all[:],
                in0=_bcast_ap(iota_sb[:], [iota_sb[:].ap[0], [0, NB], [1, G]]),
                in1=_bcast_ap(blm, [blm.ap[0], [blm.ap[1][0], NB], [0, G]]),
                op=mybir.AluOpType.is_equal,
            )
            conv_layer((h1fullA[:], h1fullB[:]),
                       [(h1shardA[:], 0, ABL), (h1shardB[:], ABLK, NSH - ABL)],
                       W2_sb, b2_sb, sink2, BF16)

            # ---- pooled tail ----
            poolT = ep.tile([F, G], F32, tag="poolT")
            nc.vector.tensor_copy(poolT[:], pool_ps[:])
            nc.sync.dma_start(pool_in[:], poolT[:])
            nc.gpsimd.collective_compute(
                "AllReduce", mybir.AluOpType.add,
                ins=[pool_in.opt()], outs=[pool_out.opt()],
                replica_groups=[list(range(C))],
            )
            poolR = ep.tile([F, G], F32, tag="poolR")
            nc.sync.dma_start(poolR[:], pool_out[:])
            nc.vector.tensor_mul(poolR[:], poolR[:], invc_sb[:])
            fc_ps = psB.tile([G, OUT], F32, tag="fc")
            nc.tensor.matmul(fc_ps[:], lhsT=poolR[:], rhs=Wfc_sb[:],
                             start=True, stop=True)
            out_sb = ep.tile([G, OUT], F32, tag="out_sb")
            nc.vector.tensor_add(out_sb[:], fc_ps[:], bfc_sb[:])
            nc.sync.dma_start(out[:], out_sb[:])

    nc.compile()
    return nc


def _in_maps(plan, per_core, invc, x, W1, b1, W2, b2, Wfc, bfc):
    iota = np.tile(np.arange(P, dtype=np.float32), (P, 1)).astype(ml_dtypes.bfloat16)
    xf = np.ascontiguousarray(np.asarray(x, np.float32))
    xr = xf.reshape(C, NSH, F)
    xA = np.ascontiguousarray(xr[:, :ABL, :].reshape(NA, F))
    xB = np.ascontiguousarray(xr[:, ABL:, :].reshape(NBB, F))
    shared = dict(
        xA=xA, xB=xB,
        iota=iota,
        W1=np.ascontiguousarray(np.asarray(W1, np.float32)),
        W2=np.ascontiguousarray(np.asarray(W2, np.float32)),
        Wfc=np.ascontiguousarray(np.asarray(Wfc, np.float32)),
        b1b=np.tile(np.asarray(b1, np.float32), (P, 1)),
        b2b=np.tile(np.asarray(b2, np.float32), (P, 1)),
        bfcb=np.tile(np.asarray(bfc, np.float32), (P, 1)),
        invc=np.tile(invc, (F, 1)),
    )
    maps = []
    for c in range(C):
        m = dict(shared)
        m.update(per_core[c])
        m["xown"] = xf[c * NSH:(c + 1) * NSH]
        maps.append({k: np.ascontiguousarray(v) for k, v in m.items()})
    return maps


_RUN_KWARGS = {}


def kernel(x, src, dst, batch, W1, b1, W2, b2, Wfc, bfc):
    plan, per_core, invc = _preprocess(src, dst, batch)
    nc = _build(plan)
    maps = _in_maps(plan, per_core, invc, x, W1, b1, W2, b2, Wfc, bfc)
    res = bass_utils.run_bass_kernel_spmd(
        nc, maps, core_ids=list(range(C)), **_RUN_KWARGS
    )
    kernel.last_results = res
    return np.asarray(res.results[0]["out"], np.float32)
